# revision 38
# baseline (speedup 1.0000x reference)
"""Causal attention head (B=4, T=4096, D=1024, H=64) on 8 TRN2 NeuronCores.

Sharding: 2 cores per batch element; core role r in {0,1} owns the
interleaved query rows {2m + r}. SPMD-uniform instruction stream: role
differences are pushed into host-side data. Core r's x^T columns are
stored pair-swapped (slot j holds global row j^r), so "even slots"
always denote the core's own query rows and the Q projection can read
stride-2 columns of x^T directly -- no separate xtq load. Causal masks
are host-computed per core with the same permutation.

Device program (per core):
  - load x^T [D, T] bf16 in 8 column slices (sync queue)
  - per slice s: K|V projection (full 512 cols), 4 PE transposes of V
    tiles batched into one copy, Q projection (256 own-row cols, read
    strided from x^T)
  - attention in 4 query-column chunks of 512: A=[0,512) after s1,
    B=[512,1024) after s3, C=[1024,1536) after s5, D=[1536,2048) after
    s7. Each chunk sweeps key-tile pairs w ascending: two score matmuls
    into one [128, 2*512] PSUM strip-pair, one exp (ScalarE, scale=1/8,
    bf16 out), one causal mask multiply on the diagonal pair, two
    context matmuls accumulating ctx^T [65, 512] (V tiles carry a ones
    column: row 64 = softmax denominator)
  - per-chunk epilogue: ctx PSUM -> SBUF copy + y DMA on GpSimd
Host: shard/cast/permute inputs, gather + divide num/den, re-interleave.
"""

import numpy as np
import ml_dtypes

import concourse.tile as tile
import concourse.mybir as mybir
from concourse import bacc
from concourse.bass_utils import run_bass_kernel_spmd

BF16 = ml_dtypes.bfloat16
F32 = np.float32

B, T, D, H = 4, 4096, 1024, 64
TL = 2048          # local query columns per core
N_CORES = 8
NKT = T // 128     # 32 key tiles
NPAIR = NKT // 2   # 16 key-tile pairs
DCH = D // 128     # 8 contraction chunks
NSL = 8            # x^T column slices of 512
DT_BF = mybir.dt.bfloat16
DT_F32 = mybir.dt.float32
EXP = mybir.ActivationFunctionType.Exp
MUL = mybir.AluOpType.mult


def _build():
    nc = bacc.Bacc("TRN2", target_bir_lowering=False, debug=False,
                   num_devices=N_CORES)

    # xt in DRAM is slice-major [s, p, d, t]: per (slice, partition) one
    # contiguous 8 KiB run -> large DMA packets, low descriptor count.
    xt = nc.dram_tensor("xt", [NSL, 128, DCH * 512], DT_BF,
                        kind="ExternalInput").ap()
    wkv = nc.dram_tensor("wkv", [D, 128], DT_BF, kind="ExternalInput").ap()
    wq = nc.dram_tensor("wq", [D, H], DT_BF, kind="ExternalInput").ap()
    masks = nc.dram_tensor("masks", [128, 256], DT_BF, kind="ExternalInput").ap()
    identb = nc.dram_tensor("identb", [128, 64], DT_BF, kind="ExternalInput").ap()
    y = nc.dram_tensor("y", [65, TL], DT_F32, kind="ExternalOutput").ap()

    with tile.TileContext(nc) as tc:
        _body(nc, tc, xt, wkv, wq, masks, identb, y)

    nc.compile()
    return nc


def _body(nc, tc, xt, wkv, wq, masks, identb, y):
    from contextlib import ExitStack

    es = ExitStack()
    with es:
        pp = es.enter_context(tc.tile_pool(name="persist", bufs=1))
        # slice-major SBUF layout [p][s][d][512] matching the DRAM layout
        xt_sb = pp.tile([128, NSL * DCH * 512], DT_BF)
        wkv_sb = pp.tile([128, DCH * 128], DT_BF)
        wq_sb = pp.tile([128, DCH * H], DT_BF)
        masks_sb = pp.tile([128, 256], DT_BF)
        identb_sb = pp.tile([128, 64], DT_BF)
        kvT_sb = pp.tile([128, T], DT_BF)       # rows 0:64 = K^T, 64:128 = V^T
        qT_sb = pp.tile([64, TL], DT_BF)
        vones_sb = pp.tile([128, NKT * 65], DT_BF)  # V tiles + ones col

        # small constants on the GpSimd SWDGE queue (slow; nothing urgent)
        nc.gpsimd.dma_start(identb_sb[:], identb[:])
        nc.gpsimd.dma_start(masks_sb[:], masks[:])

        nc.vector.memset(vones_sb[:], 1.0)

        # sync queue: weights first (they unblock the PE warmup), then the
        # x^T slices in order. Slice s covers global columns [512s, 512s+512),
        # 8 KiB contiguous per partition on both sides. Slice 0 is split in
        # half (d-chunks 0-3 / 4-7) so the first projections start sooner.
        nc.sync.dma_start(wkv_sb.rearrange("p (d t) -> p d t", t=128),
                          wkv.rearrange("(d p) t -> p d t", p=128))
        nc.sync.dma_start(wq_sb.rearrange("p (d t) -> p d t", t=H),
                          wq.rearrange("(d p) t -> p d t", p=128))
        nc.sync.dma_start(xt_sb[:, 0:2048], xt[0][:, 0:2048])
        nc.sync.dma_start(xt_sb[:, 2048:4096], xt[0][:, 2048:4096])
        for s in range(1, NSL):
            nc.sync.dma_start(xt_sb[:, s * 4096:(s + 1) * 4096], xt[s])

        xt_kv = xt_sb.rearrange("p (s d t) -> p s d t", s=NSL, d=DCH)

        proj_pool = es.enter_context(
            tc.tile_pool(name="proj", bufs=1, space="PSUM"))
        strip_pool = es.enter_context(
            tc.tile_pool(name="strips", bufs=2, space="PSUM"))
        ctx_pool = es.enter_context(
            tc.tile_pool(name="ctx", bufs=2, space="PSUM"))
        pt_pool = es.enter_context(tc.tile_pool(name="pt", bufs=4))
        cs_pool = es.enter_context(tc.tile_pool(name="cs", bufs=2))

        def kv_proj(s):
            pkv = proj_pool.tile([128, 512], DT_F32, name=f"pkv{s}", tag="pkv")
            for d in range(DCH):
                nc.tensor.matmul(
                    pkv[:],
                    lhsT=wkv_sb[:, d * 128:(d + 1) * 128],
                    rhs=xt_kv[:, s, d, :],
                    start=(d == 0), stop=(d == DCH - 1))
            nc.vector.tensor_copy(kvT_sb[:, s * 512:(s + 1) * 512], pkv[:])
            pv = proj_pool.tile([128, 256], DT_BF, name=f"pv{s}", tag="pv")
            for i in range(4):
                t = 4 * s + i
                nc.tensor.transpose(pv[:, i * 64:(i + 1) * 64],
                                    kvT_sb[64:128, t * 128:(t + 1) * 128],
                                    identb_sb[64:128, :])
            # one batched copy into the strided vones slots
            vdst = vones_sb.rearrange("p (j c) -> p j c", c=65)
            nc.vector.tensor_copy(
                vdst[:, 4 * s:4 * s + 4, 0:64],
                pv.rearrange("p (j c) -> p j c", c=64))

        def q_proj(s):
            """Q projection for local cols [256s, 256s+256) (slice s)."""
            pq = proj_pool.tile([64, 256], DT_F32, name=f"pq{s}", tag="pv")
            for d in range(DCH):
                # even slots of (s, d) block: clean 1 KiB byte range per MM
                blk = xt_sb[:, (s * DCH + d) * 512:(s * DCH + d) * 512 + 512]
                nc.tensor.matmul(
                    pq[:],
                    lhsT=wq_sb[:, d * H:(d + 1) * H],
                    rhs=blk.rearrange("p (m two) -> p m two", two=2)[:, :, 0],
                    start=(d == 0), stop=(d == DCH - 1))
            nc.vector.tensor_copy(qT_sb[:, 256 * s:256 * s + 256], pq[:])

        m3 = masks_sb.rearrange("p (two n) -> p two n", two=2)

        def make_chunk(tag, c0, width, npairs, ctx, ctx_off):
            """One query-column chunk = cols [c0, c0+width). Returns the list
            of per-pair stage closures (scores, exp_mask, context)."""
            ps_t, pt_t = {}, {}

            def scores(w):
                a = max(128 * w - c0, 0)
                ps = strip_pool.tile([128, 2 * width], DT_F32,
                                     name=f"ps{tag}_{w}", tag="ps")
                ps_t[w] = ps
                for half in range(2):
                    j = 2 * w + half
                    nc.tensor.matmul(
                        ps[:, width * half + a: width * half + width],
                        lhsT=kvT_sb[0:64, j * 128:(j + 1) * 128],
                        rhs=qT_sb[:, c0 + a: c0 + width],
                        start=True, stop=True)

            def exp_mask(w):
                a = max(128 * w - c0, 0)
                ps = ps_t.pop(w)
                pt = pt_pool.tile([128, 2 * width], DT_BF,
                                  name=f"pt{tag}_{w}", tag="pt")
                pt_t[w] = pt
                ps3 = ps.rearrange("p (two n) -> p two n", two=2)
                pt3 = pt.rearrange("p (two n) -> p two n", two=2)
                nc.scalar.activation(pt3[:, :, a:width], ps3[:, :, a:width],
                                     EXP, bias=0.0, scale=0.125)
                if c0 <= 128 * w < c0 + width:  # diagonal pair: causal mask
                    nc.vector.tensor_tensor(
                        pt3[:, :, a:a + 128], pt3[:, :, a:a + 128],
                        m3[:, :, :], MUL)

            def context(w):
                a = max(128 * w - c0, 0)
                pt = pt_t.pop(w)
                for half in range(2):
                    j = 2 * w + half
                    nc.tensor.matmul(
                        ctx[:, ctx_off + a: ctx_off + width],
                        lhsT=vones_sb[:, j * 65: j * 65 + 65],
                        rhs=pt[:, width * half + a: width * half + width],
                        start=(w == 0 and half == 0),
                        stop=(w == npairs - 1 and half == 1))

            return [(scores, exp_mask, context, w) for w in range(npairs)]

        def run_pipelined(iters, filler=None):
            """Software-pipelined: iter i+1's scores+exp are emitted before
            iter i's context MMs so TensorE never idles waiting on exp.
            `filler` is a list of closures (independent TensorE work) drained
            one per iteration to plug pipeline bubbles."""
            filler = list(filler or [])
            iters[0][0](iters[0][3])
            iters[0][1](iters[0][3])
            for i in range(len(iters)):
                if i + 1 < len(iters):
                    iters[i + 1][0](iters[i + 1][3])
                    iters[i + 1][1](iters[i + 1][3])
                if filler:
                    filler.pop(0)()
                iters[i][2](iters[i][3])
            for f in filler:
                f()

        def sweep(c, npairs):
            """512-wide chunk c, plus epilogue."""
            c0 = 512 * c
            ctx = ctx_pool.tile([65, 512], DT_F32, name=f"ctx{c}", tag="ctx")
            run_pipelined(make_chunk(c, c0, 512, npairs, ctx, 0))
            cs = cs_pool.tile([65, 512], DT_F32, name=f"cs{c}", tag="cs")
            nc.vector.tensor_copy(cs[:], ctx[:])
            nc.gpsimd.dma_start(y[:, c0:c0 + 512], cs[:])

        def warmup(n):
            """Throwaway matmuls on the weight tile to lift the HAM clock
            gate (1.2 -> 2.4 GHz needs ~3.4us of sustained PE activity)
            while the first x slice is still in flight."""
            pw = strip_pool.tile([128, 512], DT_F32, name="pwarm", tag="ps")
            for _ in range(n):
                nc.tensor.matmul(pw[:], lhsT=wkv_sb[:, 0:128],
                                 rhs=wkv_sb[:, 0:512], start=True, stop=True)

        # ---- schedule: projections interleaved with attention sweeps ----
        warmup(4)
        kv_proj(0); q_proj(0); kv_proj(1); q_proj(1)
        sweep(0, 4)
        kv_proj(2); q_proj(2); kv_proj(3); q_proj(3)
        sweep(1, 8)
        kv_proj(4); q_proj(4); kv_proj(5); q_proj(5)
        sweep(2, 12)
        # kv6/kv7 are held back (wait-pinned) to the sweep-C/D boundary:
        # they fill TensorE pipeline bubbles there so the HAM clock gate
        # never sees an idle window and the tail stays at 2.4 GHz. Their
        # V tiles are only consumed by sweep-D pairs 12-15, far later.
        q_proj(6); q_proj(7)
        with tc.tile_wait_until(0.046):
            kv_proj(6)
        with tc.tile_wait_until(0.052):
            kv_proj(7)
        sweep(3, 16)


def _host_prep(inputs):
    x = np.asarray(inputs["x"], dtype=F32)
    Wk = np.asarray(inputs["Wk"], dtype=F32)
    Wq = np.asarray(inputs["Wq"], dtype=F32)
    Wv = np.asarray(inputs["Wv"], dtype=F32)

    wkv = np.ascontiguousarray(np.concatenate([Wk, Wv], axis=1)).astype(BF16)
    wq = np.ascontiguousarray(Wq).astype(BF16)
    identb = np.zeros((128, 64), dtype=F32)
    identb[64:128, :] = np.eye(64, dtype=F32)
    identb = identb.astype(BF16)

    ii = np.arange(128)[None, :]   # query col within diag tile
    cc = np.arange(128)[:, None]   # key slot within tile
    slots = np.arange(T)
    in_maps = []
    for c in range(N_CORES):
        b, r = c // 2, c % 2
        perm = slots ^ r           # slot j holds global row j^r
        xt2 = x[b].T[:, perm].astype(BF16)          # [D, T], d-major rows
        # -> slice-major [s, p, d, t512]
        xt_np = np.ascontiguousarray(
            xt2.reshape(DCH, 128, NSL, 512).transpose(2, 1, 0, 3))
        gk = cc ^ r                # global key (mod 256) for key slot cc
        maskA = (gk <= 2 * ii + r)
        maskB = (gk + 128 <= 2 * ii + r)
        masks_np = np.concatenate([maskA, maskB], axis=1).astype(BF16)
        in_maps.append(dict(xt=xt_np, wkv=wkv, wq=wq,
                            masks=masks_np, identb=identb))
    return in_maps


def _gather(results):
    out = np.zeros((B, T, H), dtype=F32)
    m = np.arange(TL)
    for c in range(N_CORES):
        b, r = c // 2, c % 2
        yc = results[c]["y"]  # [65, TL]: rows 0:64 = ctx^T, row 64 = denom
        out[b, 2 * m + r] = (yc[:64, :] / yc[64:65, :]).T
    return out


_NC_CACHE = []


def _execute(inputs, trace=False):
    if not _NC_CACHE:
        _NC_CACHE.append(_build())
    nc = _NC_CACHE[0]
    in_maps = _host_prep(inputs)
    res = run_bass_kernel_spmd(nc, in_maps, core_ids=list(range(N_CORES)),
                               trace=trace)
    return _gather(res.results), res


def kernel(**inputs):
    out, _ = _execute(inputs, trace=False)
    return out


# revision 39
# speedup vs baseline: 1.0604x; 1.0604x over previous
"""Causal attention head (B=4, T=4096, D=1024, H=64) on 8 TRN2 NeuronCores.

Sharding: 2 cores per batch element; core role r in {0,1} owns the
interleaved query rows {2m + r}. SPMD-uniform instruction stream: role
differences are pushed into host-side data. Core r's x^T columns are
stored pair-swapped (slot j holds global row j^r), so "even slots"
always denote the core's own query rows and the Q projection can read
stride-2 columns of x^T directly -- no separate xtq load. Causal masks
are host-computed per core with the same permutation.

Device program (per core):
  - load x^T [D, T] bf16 in 8 column slices (sync queue)
  - per slice s: K|V projection (full 512 cols), 4 PE transposes of V
    tiles batched into one copy, Q projection (256 own-row cols, read
    strided from x^T)
  - attention in 4 query-column chunks of 512: A=[0,512) after s1,
    B=[512,1024) after s3, C=[1024,1536) after s5, D=[1536,2048) after
    s7. Each chunk sweeps key-tile pairs w ascending: two score matmuls
    into one [128, 2*512] PSUM strip-pair, one exp (ScalarE, scale=1/8,
    bf16 out), one causal mask multiply on the diagonal pair, two
    context matmuls accumulating ctx^T [65, 512] (V tiles carry a ones
    column: row 64 = softmax denominator)
  - per-chunk epilogue: ctx PSUM -> SBUF copy + y DMA on GpSimd
Host: shard/cast/permute inputs, gather + divide num/den, re-interleave.
"""

import numpy as np
import ml_dtypes

import concourse.tile as tile
import concourse.mybir as mybir
from concourse import bacc
from concourse.bass_utils import run_bass_kernel_spmd

BF16 = ml_dtypes.bfloat16
F32 = np.float32

B, T, D, H = 4, 4096, 1024, 64
TL = 2048          # local query columns per core
N_CORES = 8
NKT = T // 128     # 32 key tiles
NPAIR = NKT // 2   # 16 key-tile pairs
DCH = D // 128     # 8 contraction chunks
NSL = 8            # x^T column slices of 512
DT_BF = mybir.dt.bfloat16
DT_F32 = mybir.dt.float32
EXP = mybir.ActivationFunctionType.Exp
MUL = mybir.AluOpType.mult


def _build():
    nc = bacc.Bacc("TRN2", target_bir_lowering=False, debug=False,
                   num_devices=N_CORES)

    # xt in DRAM is slice-major [s, p, d, t]: per (slice, partition) one
    # contiguous 8 KiB run -> large DMA packets, low descriptor count.
    xt = nc.dram_tensor("xt", [NSL, 128, DCH * 512], DT_BF,
                        kind="ExternalInput").ap()
    wkv = nc.dram_tensor("wkv", [D, 128], DT_BF, kind="ExternalInput").ap()
    wq = nc.dram_tensor("wq", [D, H], DT_BF, kind="ExternalInput").ap()
    masks = nc.dram_tensor("masks", [128, 256], DT_BF, kind="ExternalInput").ap()
    identb = nc.dram_tensor("identb", [128, 64], DT_BF, kind="ExternalInput").ap()
    y = nc.dram_tensor("y", [65, TL], DT_F32, kind="ExternalOutput").ap()

    with tile.TileContext(nc) as tc:
        _body(nc, tc, xt, wkv, wq, masks, identb, y)

    nc.compile()
    return nc


def _body(nc, tc, xt, wkv, wq, masks, identb, y):
    from contextlib import ExitStack

    es = ExitStack()
    with es:
        pp = es.enter_context(tc.tile_pool(name="persist", bufs=1))
        # slice-major SBUF layout [p][s][d][512] matching the DRAM layout
        xt_sb = pp.tile([128, NSL * DCH * 512], DT_BF)
        wkv_sb = pp.tile([128, DCH * 128], DT_BF)
        wq_sb = pp.tile([128, DCH * H], DT_BF)
        masks_sb = pp.tile([128, 256], DT_BF)
        identb_sb = pp.tile([128, 64], DT_BF)
        kvT_sb = pp.tile([128, T], DT_BF)       # rows 0:64 = K^T, 64:128 = V^T
        qT_sb = pp.tile([64, TL], DT_BF)
        vones_sb = pp.tile([128, NKT * 65], DT_BF)  # V tiles + ones col

        # small constants on the GpSimd SWDGE queue (slow; nothing urgent)
        nc.gpsimd.dma_start(identb_sb[:], identb[:])
        nc.gpsimd.dma_start(masks_sb[:], masks[:])

        nc.vector.memset(vones_sb[:], 1.0)

        # sync queue: weights first (they unblock the PE warmup), then the
        # x^T slices in order. Slice s covers global columns [512s, 512s+512),
        # 8 KiB contiguous per partition on both sides. Slice 0 is split in
        # half (d-chunks 0-3 / 4-7) so the first projections start sooner.
        nc.sync.dma_start(wkv_sb.rearrange("p (d t) -> p d t", t=128),
                          wkv.rearrange("(d p) t -> p d t", p=128))
        nc.sync.dma_start(wq_sb.rearrange("p (d t) -> p d t", t=H),
                          wq.rearrange("(d p) t -> p d t", p=128))
        nc.sync.dma_start(xt_sb[:, 0:2048], xt[0][:, 0:2048])
        nc.sync.dma_start(xt_sb[:, 2048:4096], xt[0][:, 2048:4096])
        for s in range(1, NSL):
            nc.sync.dma_start(xt_sb[:, s * 4096:(s + 1) * 4096], xt[s])

        xt_kv = xt_sb.rearrange("p (s d t) -> p s d t", s=NSL, d=DCH)

        proj_pool = es.enter_context(
            tc.tile_pool(name="proj", bufs=1, space="PSUM"))
        strip_pool = es.enter_context(
            tc.tile_pool(name="strips", bufs=2, space="PSUM"))
        ctx_pool = es.enter_context(
            tc.tile_pool(name="ctx", bufs=1, space="PSUM"))
        pt_pool = es.enter_context(tc.tile_pool(name="pt", bufs=4))
        cs_pool = es.enter_context(tc.tile_pool(name="cs", bufs=2))

        def kv_proj(s):
            pkv = proj_pool.tile([128, 512], DT_F32, name=f"pkv{s}", tag="pkv")
            for d in range(DCH):
                nc.tensor.matmul(
                    pkv[:],
                    lhsT=wkv_sb[:, d * 128:(d + 1) * 128],
                    rhs=xt_kv[:, s, d, :],
                    start=(d == 0), stop=(d == DCH - 1))
            nc.vector.tensor_copy(kvT_sb[:, s * 512:(s + 1) * 512], pkv[:])
            pv = proj_pool.tile([128, 256], DT_BF, name=f"pv{s}", tag="pv")
            for i in range(4):
                t = 4 * s + i
                nc.tensor.transpose(pv[:, i * 64:(i + 1) * 64],
                                    kvT_sb[64:128, t * 128:(t + 1) * 128],
                                    identb_sb[64:128, :])
            # one batched copy into the strided vones slots
            vdst = vones_sb.rearrange("p (j c) -> p j c", c=65)
            nc.vector.tensor_copy(
                vdst[:, 4 * s:4 * s + 4, 0:64],
                pv.rearrange("p (j c) -> p j c", c=64))

        def q_proj(s):
            """Q projection for local cols [256s, 256s+256) (slice s)."""
            pq = proj_pool.tile([64, 256], DT_F32, name=f"pq{s}", tag="pq")
            for d in range(DCH):
                # even slots of (s, d) block: clean 1 KiB byte range per MM
                blk = xt_sb[:, (s * DCH + d) * 512:(s * DCH + d) * 512 + 512]
                nc.tensor.matmul(
                    pq[:],
                    lhsT=wq_sb[:, d * H:(d + 1) * H],
                    rhs=blk.rearrange("p (m two) -> p m two", two=2)[:, :, 0],
                    start=(d == 0), stop=(d == DCH - 1))
            nc.vector.tensor_copy(qT_sb[:, 256 * s:256 * s + 256], pq[:])

        m3 = masks_sb.rearrange("p (two n) -> p two n", two=2)

        def make_chunk(tag, c0, width, npairs, ctx, ctx_off):
            """One query-column chunk = cols [c0, c0+width). Returns the list
            of per-pair stage closures (scores, exp_mask, context)."""
            ps_t, pt_t = {}, {}

            def scores(w):
                a = max(128 * w - c0, 0)
                ps = strip_pool.tile([128, 2 * width], DT_F32,
                                     name=f"ps{tag}_{w}", tag="ps")
                ps_t[w] = ps
                for half in range(2):
                    j = 2 * w + half
                    nc.tensor.matmul(
                        ps[:, width * half + a: width * half + width],
                        lhsT=kvT_sb[0:64, j * 128:(j + 1) * 128],
                        rhs=qT_sb[:, c0 + a: c0 + width],
                        start=True, stop=True)

            def exp_mask(w):
                a = max(128 * w - c0, 0)
                ps = ps_t.pop(w)
                pt = pt_pool.tile([128, 2 * width], DT_BF,
                                  name=f"pt{tag}_{w}", tag="pt")
                pt_t[w] = pt
                ps3 = ps.rearrange("p (two n) -> p two n", two=2)
                pt3 = pt.rearrange("p (two n) -> p two n", two=2)
                nc.scalar.activation(pt3[:, :, a:width], ps3[:, :, a:width],
                                     EXP, bias=0.0, scale=0.125)
                if c0 <= 128 * w < c0 + width:  # diagonal pair: causal mask
                    nc.vector.tensor_tensor(
                        pt3[:, :, a:a + 128], pt3[:, :, a:a + 128],
                        m3[:, :, :], MUL)

            def context(w):
                a = max(128 * w - c0, 0)
                pt = pt_t.pop(w)
                for half in range(2):
                    j = 2 * w + half
                    nc.tensor.matmul(
                        ctx[:, ctx_off + a: ctx_off + width],
                        lhsT=vones_sb[:, j * 65: j * 65 + 65],
                        rhs=pt[:, width * half + a: width * half + width],
                        start=(w == 0 and half == 0),
                        stop=(w == npairs - 1 and half == 1))

            return [(scores, exp_mask, context, w) for w in range(npairs)]

        def run_pipelined(iters, filler=None):
            """Software-pipelined: iter i+1's scores+exp are emitted before
            iter i's context MMs so TensorE never idles waiting on exp.
            `filler` is a list of closures (independent TensorE work) drained
            one per iteration to plug pipeline bubbles."""
            filler = list(filler or [])
            iters[0][0](iters[0][3])
            iters[0][1](iters[0][3])
            for i in range(len(iters)):
                if i + 1 < len(iters):
                    iters[i + 1][0](iters[i + 1][3])
                    iters[i + 1][1](iters[i + 1][3])
                if filler:
                    filler.pop(0)()
                iters[i][2](iters[i][3])
            for f in filler:
                f()

        def sweep(c, npairs):
            """512-wide chunk c, plus epilogue."""
            c0 = 512 * c
            ctx = ctx_pool.tile([65, 512], DT_F32, name=f"ctx{c}", tag="ctx")
            run_pipelined(make_chunk(c, c0, 512, npairs, ctx, 0))
            cs = cs_pool.tile([65, 512], DT_F32, name=f"cs{c}", tag="cs")
            nc.vector.tensor_copy(cs[:], ctx[:])
            nc.gpsimd.dma_start(y[:, c0:c0 + 512], cs[:])

        def warmup(n):
            """Throwaway matmuls on the weight tile to lift the HAM clock
            gate (1.2 -> 2.4 GHz needs ~3.4us of sustained PE activity)
            while the first x slice is still in flight."""
            pw = strip_pool.tile([128, 512], DT_F32, name="pwarm", tag="ps")
            for _ in range(n):
                nc.tensor.matmul(pw[:], lhsT=wkv_sb[:, 0:128],
                                 rhs=wkv_sb[:, 0:512], start=True, stop=True)

        # ---- schedule: projections interleaved with attention sweeps ----
        warmup(4)
        kv_proj(0); q_proj(0); kv_proj(1); q_proj(1)
        sweep(0, 4)
        kv_proj(2); q_proj(2); kv_proj(3); q_proj(3)
        sweep(1, 8)
        kv_proj(4); q_proj(4); kv_proj(5); q_proj(5)
        sweep(2, 12)
        # kv6/kv7 are held back (wait-pinned) to the sweep-C/D boundary:
        # they fill TensorE pipeline bubbles there so the HAM clock gate
        # never sees an idle window and the tail stays at 2.4 GHz. Their
        # V tiles are only consumed by sweep-D pairs 12-15, far later.
        q_proj(6); q_proj(7)
        with tc.tile_wait_until(0.046):
            kv_proj(6)
        with tc.tile_wait_until(0.052):
            kv_proj(7)
        sweep(3, 16)


def _host_prep(inputs):
    x = np.asarray(inputs["x"], dtype=F32)
    Wk = np.asarray(inputs["Wk"], dtype=F32)
    Wq = np.asarray(inputs["Wq"], dtype=F32)
    Wv = np.asarray(inputs["Wv"], dtype=F32)

    wkv = np.ascontiguousarray(np.concatenate([Wk, Wv], axis=1)).astype(BF16)
    wq = np.ascontiguousarray(Wq).astype(BF16)
    identb = np.zeros((128, 64), dtype=F32)
    identb[64:128, :] = np.eye(64, dtype=F32)
    identb = identb.astype(BF16)

    ii = np.arange(128)[None, :]   # query col within diag tile
    cc = np.arange(128)[:, None]   # key slot within tile
    slots = np.arange(T)
    in_maps = []
    for c in range(N_CORES):
        b, r = c // 2, c % 2
        perm = slots ^ r           # slot j holds global row j^r
        xt2 = x[b].T[:, perm].astype(BF16)          # [D, T], d-major rows
        # -> slice-major [s, p, d, t512]
        xt_np = np.ascontiguousarray(
            xt2.reshape(DCH, 128, NSL, 512).transpose(2, 1, 0, 3))
        gk = cc ^ r                # global key (mod 256) for key slot cc
        maskA = (gk <= 2 * ii + r)
        maskB = (gk + 128 <= 2 * ii + r)
        masks_np = np.concatenate([maskA, maskB], axis=1).astype(BF16)
        in_maps.append(dict(xt=xt_np, wkv=wkv, wq=wq,
                            masks=masks_np, identb=identb))
    return in_maps


def _gather(results):
    out = np.zeros((B, T, H), dtype=F32)
    m = np.arange(TL)
    for c in range(N_CORES):
        b, r = c // 2, c % 2
        yc = results[c]["y"]  # [65, TL]: rows 0:64 = ctx^T, row 64 = denom
        out[b, 2 * m + r] = (yc[:64, :] / yc[64:65, :]).T
    return out


_NC_CACHE = []


def _execute(inputs, trace=False):
    if not _NC_CACHE:
        _NC_CACHE.append(_build())
    nc = _NC_CACHE[0]
    in_maps = _host_prep(inputs)
    res = run_bass_kernel_spmd(nc, in_maps, core_ids=list(range(N_CORES)),
                               trace=trace)
    return _gather(res.results), res


def kernel(**inputs):
    out, _ = _execute(inputs, trace=False)
    return out
